# revision 8
# baseline (speedup 1.0000x reference)
"""Trainium2 Bass kernel for nn_Aggregator (Linear -> LayerNorm -> segment mean).

Full inputs in, full output out. Internally:
  - batch is sorted, so shard rows at segment boundaries across 8 cores
    (each core owns a disjoint range of 2048 segments -> no all-reduce).
  - Host folds LayerNorm mean-centering into W/b:  W'' = W - colmean(W),
    b'' = b - mean(b), so h_c = W''x + b'' is exactly mean-centered, and
    ln_w/ln_b commute with the segment mean (applied once per window).
  - Device, per 128-token tile (tokens on partitions; x pre-transposed on
    host so xT tiles load contiguously):
      psum_h = x_tile @ W''^T                  (PE matmul, lhsT = xT tile)
      h_c    = psum_h + b''                    (DVE tensor_tensor, PSUM->SBUF)
      ssq    = sum_o h_c_o^2                   (ACT Square with accum_out)
      s      = Sqrt(ssq/128 + eps)             (ACT)
      rstd   = 1/s                             (DVE reciprocal)
      sel    = (iota_w == batch_tile) * rstd   (DVE dual-op tensor_scalar)
      psum_seg[128 seg, 128] += sel^T @ h_c    (PE matmul, PSUM-accumulated
                                                over all tiles of a
                                                128-segment window)
    Window drain: out = psum_seg/max(cnt,1) * ln_w + ln_b*(cnt>0); counts
    come from a host-side bincount of the (index-only) batch tensor.
  - Padding tokens carry batch id -1 -> selector row all zero -> inert.
"""

import math
import numpy as np

P = 128
D = 128          # IN_DIM == OUT_DIM
NSEG = 16384
NCORES = 8
SEG_PER_CORE = NSEG // NCORES   # 2048
NWIN = SEG_PER_CORE // P        # 16 windows of 128 segments per core
EPS = 1e-5
CHUNK = 32                      # tiles per x-chunk DMA (32*128*128*4 = 2 MiB)


def _build_program(TW, nwin, seg_per_core):
    import concourse.tile as tile
    from concourse import bacc, mybir

    f32 = mybir.dt.float32
    AF = mybir.ActivationFunctionType
    OP = mybir.AluOpType

    NTILES = nwin * TW
    NTOK = NTILES * P
    # packed const layout (columns)
    OW = 0                      # wa: W''^T               [128, 128]
    OB = OW + D                 # brep: b'' replicated    [128, 128]
    OLW = OB + D                # ln_w replicated         [128, 128]
    OLB = OLW + D               # ln_b replicated         [128, 128]
    OCW = OLB + D               # counts per window       [128, nwin]
    OIO = OCW + nwin            # iota (abs seg ids)      [128, nwin*128]
    OBT = OIO + nwin * P        # batch per tile          [128, NTILES]
    CC = OBT + NTILES

    nc = bacc.Bacc(None, target_bir_lowering=False)
    xt = nc.dram_tensor("xt", [P, NTOK], f32, kind="ExternalInput")
    cst = nc.dram_tensor("cst", [P, CC], f32, kind="ExternalInput")
    outd = nc.dram_tensor("out", [seg_per_core, D], f32, kind="ExternalOutput")

    with tile.TileContext(nc) as tc:
        with (
            tc.tile_pool(name="const", bufs=1) as cpool,
            tc.tile_pool(name="xch", bufs=3) as xpool,
            tc.tile_pool(name="h", bufs=4) as hpool,
            tc.tile_pool(name="sel", bufs=4) as selpool,
            tc.tile_pool(name="sq", bufs=2) as sqpool,
            tc.tile_pool(name="mini", bufs=6) as minipool,
            tc.tile_pool(name="outp", bufs=2) as outpool,
            tc.tile_pool(name="ph", bufs=4, space="PSUM") as phpool,
            tc.tile_pool(name="ps", bufs=2, space="PSUM") as pspool,
        ):
            c_sb = cpool.tile([P, CC], f32, tag="cst")
            nc.sync.dma_start(c_sb[:], cst[:])
            wa_sb = c_sb[:, OW: OW + D]
            brep = c_sb[:, OB: OB + D]
            lnw_sb = c_sb[:, OLW: OLW + D]
            lnb_sb = c_sb[:, OLB: OLB + D]
            sbias = cpool.tile([P, 1], f32, tag="sbias")
            nc.gpsimd.memset(sbias[:], float(EPS))

            xch = None
            for w in range(nwin):
                psum_seg = pspool.tile([P, D], f32, tag="pseg")
                for j in range(TW):
                    jj = w * TW + j
                    if jj % CHUNK == 0:
                        csz = min(CHUNK, NTILES - jj) * P
                        xch = xpool.tile([P, csz], f32, tag="xch")
                        nc.sync.dma_start(xch[:], xt[:, jj * P: jj * P + csz])
                    k = (jj % CHUNK) * P
                    psum_h = phpool.tile([P, D], f32, tag="ph")
                    nc.tensor.matmul(
                        psum_h[:], xch[:, k: k + P], wa_sb,
                        start=True, stop=True,
                    )
                    # h_c = W''x + b''  (PSUM -> SBUF)
                    h_c = hpool.tile([P, D], f32, tag="h")
                    nc.vector.tensor_tensor(h_c[:], psum_h[:], brep, op=OP.add)
                    # ssq = sum_o h_c^2 ; s = sqrt(ssq/128 + eps) ; rstd = 1/s
                    sqd = sqpool.tile([P, D], f32, tag="sq")
                    ssq = minipool.tile([P, 1], f32, tag="ssq")
                    nc.scalar.activation(
                        sqd[:], h_c[:], AF.Square, accum_out=ssq[:],
                    )
                    s = minipool.tile([P, 1], f32, tag="s")
                    nc.scalar.activation(
                        s[:], ssq[:], AF.Sqrt, scale=1.0 / D, bias=sbias[:],
                    )
                    rstd = minipool.tile([P, 1], f32, tag="rstd")
                    nc.vector.reciprocal(rstd[:], s[:])
                    # selector = (iota_w == batch_j) * rstd
                    sel = selpool.tile([P, P], f32, tag="sel")
                    nc.vector.tensor_scalar(
                        sel[:], c_sb[:, OIO + w * P: OIO + (w + 1) * P],
                        c_sb[:, OBT + jj: OBT + jj + 1], rstd[:],
                        OP.is_equal, OP.mult,
                    )
                    nc.tensor.matmul(
                        psum_seg[:], sel[:], h_c[:],
                        start=(j == 0), stop=(j == TW - 1),
                    )
                # ---- drain window w ----
                cmax = minipool.tile([P, 1], f32, tag="cmax")
                nc.vector.tensor_scalar_max(
                    cmax[:], c_sb[:, OCW + w: OCW + w + 1], 1.0)
                recip = minipool.tile([P, 1], f32, tag="recip")
                nc.vector.reciprocal(recip[:], cmax[:])
                ind = minipool.tile([P, 1], f32, tag="ind")
                nc.vector.tensor_scalar_min(
                    ind[:], c_sb[:, OCW + w: OCW + w + 1], 1.0)
                out1 = outpool.tile([P, D], f32, tag="out1")
                nc.vector.tensor_scalar(
                    out1[:], psum_seg[:], recip[:], None, OP.mult,
                )
                nc.vector.tensor_tensor(out1[:], out1[:], lnw_sb, op=OP.mult)
                out2 = outpool.tile([P, D], f32, tag="out2")
                nc.vector.tensor_scalar(out2[:], lnb_sb, ind[:], None, OP.mult)
                nc.vector.tensor_tensor(out1[:], out1[:], out2[:], op=OP.add)
                nc.sync.dma_start(outd[w * P: (w + 1) * P, :], out1[:])
    return nc


def _prepare(x, batch, W, b, ln_w, ln_b, nwin=NWIN, ncores=NCORES):
    """Host-side shard/layout prep. Returns (in_maps, TW)."""
    x = np.ascontiguousarray(np.asarray(x, dtype=np.float32))
    batch = np.asarray(batch).astype(np.int64)
    W = np.asarray(W, dtype=np.float32)
    b = np.asarray(b, dtype=np.float32)
    ln_w = np.asarray(ln_w, dtype=np.float32)
    ln_b = np.asarray(ln_b, dtype=np.float32)

    nseg = ncores * nwin * P
    Wpp = (W - W.mean(axis=0, keepdims=True)).astype(np.float32)
    bpp = (b - b.mean()).astype(np.float32)

    # token ranges of every 128-segment window (global windows = ncores*nwin)
    edges = np.searchsorted(batch, np.arange(0, nseg + 1, P))
    wcounts = np.diff(edges)
    TW = max(1, int(math.ceil(wcounts.max() / P)))
    NTILES = nwin * TW
    NTOK = NTILES * P

    OW = 0
    OB = OW + D
    OLW = OB + D
    OLB = OLW + D
    OCW = OLB + D
    OIO = OCW + nwin
    OBT = OIO + nwin * P
    CC = OBT + NTILES

    in_maps = []
    for c in range(ncores):
        xt_np = np.zeros((P, NTOK), np.float32)
        bt_np = np.full((NTILES * P,), -1.0, np.float32)
        for w in range(nwin):
            g = c * nwin + w
            s, e = int(edges[g]), int(edges[g + 1])
            n = e - s
            col0 = w * TW * P
            if n:
                xt_np[:, col0: col0 + n] = x[s:e].T
                bt_np[col0: col0 + n] = batch[s:e].astype(np.float32)
        base = c * nwin * P
        rs, re = int(edges[c * nwin]), int(edges[(c + 1) * nwin])
        cnts = np.bincount(
            (batch[rs:re] - base).astype(np.int64), minlength=nwin * P
        ).astype(np.float32)

        cst = np.empty((P, CC), np.float32)
        cst[:, OW: OW + D] = Wpp.T
        cst[:, OB: OB + D] = bpp[None, :]
        cst[:, OLW: OLW + D] = ln_w[None, :]
        cst[:, OLB: OLB + D] = ln_b[None, :]
        cst[:, OCW: OCW + nwin] = cnts.reshape(nwin, P).T
        cst[:, OIO: OIO + nwin * P] = (
            base + np.arange(nwin * P, dtype=np.float32))[None, :]
        cst[:, OBT: OBT + NTILES] = bt_np.reshape(NTILES, P).T
        in_maps.append({"xt": xt_np, "cst": cst})
    return in_maps, TW


TRACE = False          # set True (e.g. from test.py) to neuron-profile the run
TRACE_DIR = None
LAST = None            # BassKernelResults of the most recent kernel() call


def kernel(x, batch, W, b, ln_w, ln_b):
    from concourse.bass_utils import run_bass_kernel_spmd

    in_maps, TW = _prepare(x, batch, W, b, ln_w, ln_b)
    nc = _build_program(TW, NWIN, SEG_PER_CORE)
    nc.finalize()
    kw = {}
    if TRACE:
        kw = dict(trace=True, tmpdir=TRACE_DIR)
    res = run_bass_kernel_spmd(nc, in_maps, list(range(NCORES)), **kw)
    global LAST
    LAST = res
    out = np.concatenate(
        [res.results[c]["out"] for c in range(NCORES)], axis=0
    ).astype(np.float32)
    return out
